# revision 14
# baseline (speedup 1.0000x reference)
"""Causal self-attention with RoPE on 8 TRN2 NeuronCores.

Sharding: core c -> (batch b = c//4, head-group g = c%4; 4 heads of 128 each).
Tensor-parallel over heads x data-parallel over batch. After per-head
attention, the 4 cores of a batch AllGather their y^T shards, then each
core computes a disjoint 512-column slice of the output projection.

Layouts (all chosen so no on-chip transposes are ever needed):
  xT   [D, S]   = x[b].T                      (host-transposed)
  Q^T,K^T [128, S] per head  (from matmul: lhsT=W-block, rhs=xT)
  V    [S, 512] token-major  (from matmul: lhsT=xT-tile, rhs=Wv)
  S^T  [j, i] scores blocks -> softmax sums via ones-matmul on PE
  O^T  [c, i] accumulated in PSUM, normalized by 1/rowsum afterwards
  z^T  [512, S] output slice (host transposes back)

All matmuls run in float32r (~13-bit mantissa, 4x faster than fp32 on PE).
"""
from contextlib import ExitStack

import numpy as np

import concourse.bass as bass
import concourse.tile as tile
import concourse.mybir as mybir
from concourse import bacc, bass_utils

import os as _os
B = 2
S = int(_os.environ.get("K_S", "2048"))
D = int(_os.environ.get("K_D", "2048"))
NH, HD = 16, 128
HPC = 4                 # heads per core
EL = HPC * HD           # 512: local e-width per core
CH = 512                # i-chunk / s-chunk width
NCH = S // CH           # 4
DT = D // 128           # 16 d-tiles
ROPE_THETA = 10000.0
N_CORES = 8

F32 = mybir.dt.float32
F32R = mybir.dt.float32r
AF = mybir.ActivationFunctionType


def _build():
    nc = bacc.Bacc("TRN2", target_bir_lowering=False, debug=False,
                   enable_asserts=True, num_devices=N_CORES)
    xT = nc.dram_tensor("xT", [D, S], F32R, kind="ExternalInput").ap()
    wq = nc.dram_tensor("wq", [D, EL], F32R, kind="ExternalInput").ap()
    wk = nc.dram_tensor("wk", [D, EL], F32R, kind="ExternalInput").ap()
    wv = nc.dram_tensor("wv", [D, EL], F32R, kind="ExternalInput").ap()
    wp = nc.dram_tensor("wp", [D, EL], F32R, kind="ExternalInput").ap()
    cosq = nc.dram_tensor("cosq", [HD, S], F32, kind="ExternalInput").ap()
    sinq = nc.dram_tensor("sinq", [HD, S], F32, kind="ExternalInput").ap()
    cosk = nc.dram_tensor("cosk", [HD, S], F32, kind="ExternalInput").ap()
    sink = nc.dram_tensor("sink", [HD, S], F32, kind="ExternalInput").ap()
    tri = nc.dram_tensor("tri", [128, 128], F32, kind="ExternalInput").ap()
    ones = nc.dram_tensor("ones", [128, 1], F32R, kind="ExternalInput").ap()
    onesT = nc.dram_tensor("onesT", [1, 128], F32R, kind="ExternalInput").ap()
    zT = nc.dram_tensor("zT", [EL, S], F32, kind="ExternalOutput").ap()

    xTr = xT.rearrange("(t p) s -> t p s", p=128)
    wqr = wq.rearrange("(t p) e -> t p e", p=128)
    wkr = wk.rearrange("(t p) e -> t p e", p=128)
    wvr = wv.rearrange("(t p) e -> t p e", p=128)
    wpr = wp.rearrange("(t p) e -> t p e", p=128)

    with tile.TileContext(nc) as tc, \
         nc.allow_low_precision(reason="fp32r attention"), ExitStack() as ctx:
        if True:
            vres = ctx.enter_context(tc.tile_pool(name="vres", bufs=16))
            kres = ctx.enter_context(tc.tile_pool(name="kres", bufs=4))
            cpool = ctx.enter_context(tc.tile_pool(name="const", bufs=1))
            dram = ctx.enter_context(tc.tile_pool(name="dram", bufs=1, space="DRAM"))
            ps_mm = ctx.enter_context(tc.tile_pool(name="ps_mm", bufs=2, space="PSUM"))
            ps_sc = ctx.enter_context(tc.tile_pool(name="ps_sc", bufs=3, space="PSUM"))
            ps_o = ctx.enter_context(tc.tile_pool(name="ps_o", bufs=1, space="PSUM"))
            ps_r = ctx.enter_context(tc.tile_pool(name="ps_r", bufs=1, space="PSUM"))
            ps_b = ctx.enter_context(tc.tile_pool(name="ps_b", bufs=1, space="PSUM"))

            tri_t = cpool.tile([128, 128], F32)
            nc.sync.dma_start(tri_t[:], tri)
            ones_t = cpool.tile([128, 1], F32R)
            nc.sync.dma_start(ones_t[:], ones)
            onesT_t = cpool.tile([1, 128], F32R)
            nc.sync.dma_start(onesT_t[:], onesT)

            q_spill = dram.tile([EL, S], F32R)
            y_loc = [dram.tile([EL, CH], F32R, tag=f"yl{ci}", name=f"yl{ci}")
                     for ci in range(NCH)]
            y_full = [dram.tile([D, CH], F32R, tag=f"yf{ci}", name=f"yf{ci}")
                      for ci in range(NCH)]

            v_t = [vres.tile([128, EL], F32R, tag="v", name=f"v{st}")
                   for st in range(S // 128)]
            k_t = [kres.tile([HD, S], F32R, tag="k", name=f"k{h}")
                   for h in range(HPC)]

            # ---------------- pass 1: V = x @ Wv  (token-major) -------------
            with ExitStack() as vctx:
                p1w = vctx.enter_context(tc.tile_pool(name="p1", bufs=18))
                p1x = vctx.enter_context(tc.tile_pool(name="p1x", bufs=18))
                wv_t = []
                for dt in range(DT):
                    w = p1w.tile([128, EL], F32R, tag="w", name=f"wv{dt}")
                    nc.sync.dma_start(w[:], wvr[dt])
                    wv_t.append(w)
                for sc in range(NCH):
                    xc = []
                    for dt in range(DT):
                        xt = p1x.tile([128, CH], F32R, tag="x", name=f"x{sc}_{dt}")
                        nc.sync.dma_start(xt[:], xTr[dt][:, sc * CH:(sc + 1) * CH])
                        xc.append(xt)
                    for st in range(CH // 128):
                        ps = ps_mm.tile([128, EL], F32)
                        for dt in range(DT):
                            nc.tensor.matmul(
                                ps[:], xc[dt][:, st * 128:(st + 1) * 128], wv_t[dt][:],
                                start=(dt == 0), stop=(dt == DT - 1))
                        nc.scalar.copy(v_t[sc * 4 + st][:], ps[:])

            # ------------- passes 2/3: K^T then Q^T (+RoPE), Q spilled -------
            def kq_pass(wsrc, cos_src, sin_src, is_q, tagp):
                with ExitStack() as kctx:
                    pw = kctx.enter_context(tc.tile_pool(name=f"{tagp}w", bufs=18))
                    px = kctx.enter_context(tc.tile_pool(name=f"{tagp}x", bufs=18))
                    pcs = kctx.enter_context(tc.tile_pool(name=f"{tagp}cs", bufs=2))
                    pt = kctx.enter_context(tc.tile_pool(name=f"{tagp}t", bufs=3))
                    w_t = []
                    for dt in range(DT):
                        w = pw.tile([128, EL], F32R, tag="w", name=f"{tagp}w{dt}")
                        nc.sync.dma_start(w[:], wsrc[dt])
                        w_t.append(w)
                    for sc in range(NCH):
                        xc = []
                        for dt in range(DT):
                            xt = px.tile([128, CH], F32R, tag="x",
                                         name=f"{tagp}x{sc}_{dt}")
                            nc.sync.dma_start(
                                xt[:], xTr[dt][:, sc * CH:(sc + 1) * CH])
                            xc.append(xt)
                        cs = pcs.tile([128, CH], F32, tag="cs")
                        nc.sync.dma_start(cs[:], cos_src[:, sc * CH:(sc + 1) * CH])
                        sn = pcs.tile([128, CH], F32, tag="sn")
                        nc.sync.dma_start(sn[:], sin_src[:, sc * CH:(sc + 1) * CH])
                        for h in range(HPC):
                            ps = ps_mm.tile([HD, CH], F32)
                            for dt in range(DT):
                                nc.tensor.matmul(
                                    ps[:], w_t[dt][:, h * HD:(h + 1) * HD], xc[dt][:],
                                    start=(dt == 0), stop=(dt == DT - 1))
                            pre = pt.tile([128, CH], F32, tag="pre")
                            nc.scalar.copy(pre[:], ps[:])
                            # out = pre*cos + rot(pre)*sin (sign folded into sin)
                            rot = pt.tile([128, CH], F32, tag="rot")
                            nc.sync.dma_start(rot[0:64, :], pre[64:128, :])
                            nc.sync.dma_start(rot[64:128, :], pre[0:64, :])
                            t1 = pt.tile([128, CH], F32, tag="t1")
                            nc.vector.tensor_mul(t1[:], pre[:], cs[:])
                            t2 = pt.tile([128, CH], F32, tag="rot2")
                            nc.vector.tensor_mul(t2[:], rot[:], sn[:])
                            if is_q:
                                qro = pt.tile([128, CH], F32R, tag="qro")
                                nc.vector.tensor_add(qro[:], t1[:], t2[:])
                                nc.sync.dma_start(
                                    q_spill[h * HD:(h + 1) * HD,
                                            sc * CH:(sc + 1) * CH], qro[:])
                            else:
                                nc.vector.tensor_add(
                                    k_t[h][:, sc * CH:(sc + 1) * CH], t1[:], t2[:])

            kq_pass(wkr, cosk, sink, False, "pk")
            kq_pass(wqr, cosq, sinq, True, "pq")

            # ------- pass 3: causal attention + AG + projection (pipelined) --
            with ExitStack() as actx:
                p3q = actx.enter_context(tc.tile_pool(name="p3q", bufs=5))
                p3p = actx.enter_context(tc.tile_pool(name="p3p", bufs=3))
                p3o = actx.enter_context(tc.tile_pool(name="p3o", bufs=4))
                p3y = actx.enter_context(tc.tile_pool(name="p3y", bufs=2))
                p3r = actx.enter_context(tc.tile_pool(name="p3r", bufs=2))
                p4w = actx.enter_context(tc.tile_pool(name="p4w", bufs=16))
                p4y = actx.enter_context(tc.tile_pool(name="p4y", bufs=18))
                p4z = actx.enter_context(tc.tile_pool(name="p4z", bufs=3))
                wp_t = []
                for et in range(DT):
                    w = p4w.tile([128, EL], F32R, tag="w", name=f"wp{et}")
                    nc.sync.dma_start(w[:], wpr[et])
                    wp_t.append(w)

                def proj_chunk(sc):
                    yfr = y_full[sc][:].rearrange("(t p) s -> t p s", p=128)
                    yc = []
                    for et in range(DT):
                        yt = p4y.tile([128, CH], F32R, tag="y", name=f"yg{sc}_{et}")
                        nc.sync.dma_start(yt[:], yfr[et])
                        yc.append(yt)
                    for ep in range(EL // 128):
                        ps = ps_mm.tile([128, CH], F32)
                        for et in range(DT):
                            nc.tensor.matmul(
                                ps[:], wp_t[et][:, ep * 128:(ep + 1) * 128], yc[et][:],
                                start=(et == 0), stop=(et == DT - 1))
                        zt = p4z.tile([128, CH], F32, tag="z")
                        nc.scalar.copy(zt[:], ps[:])
                        nc.sync.dma_start(
                            zT[ep * 128:(ep + 1) * 128, sc * CH:(sc + 1) * CH], zt[:])

                for ci in range(NCH):
                    qc = []
                    for h in range(HPC):
                        qt = p3q.tile([HD, CH], F32R, tag="q", name=f"q{ci}_{h}")
                        nc.sync.dma_start(
                            qt[:], q_spill[h * HD:(h + 1) * HD, ci * CH:(ci + 1) * CH])
                        qc.append(qt)
                    rall = p3r.tile([HPC, CH], F32, tag="rall")
                    o_sb = []
                    n_jt = 4 * ci + 4
                    for h in range(HPC):
                        o_ps = ps_o.tile([HD, CH], F32)
                        r_ps = ps_r.tile([1, CH], F32)
                        for jt in range(n_jt):
                            diag = jt - 4 * ci
                            off = 128 * diag if diag > 0 else 0
                            s_ps = ps_sc.tile([128, CH], F32)
                            nc.tensor.matmul(
                                s_ps[:, off:], k_t[h][:, jt * 128:(jt + 1) * 128],
                                qc[h][:, off:], start=True, stop=True)
                            p = p3p.tile([128, CH], F32R, tag="p")
                            nc.scalar.activation(p[:, off:], s_ps[:, off:], AF.Exp)
                            if diag >= 0:
                                nc.vector.tensor_mul(
                                    p[:, off:off + 128], p[:, off:off + 128], tri_t[:])
                            nc.tensor.matmul(
                                o_ps[:, off:], v_t[jt][:, h * HD:(h + 1) * HD],
                                p[:, off:], start=(jt == 0), stop=(jt == n_jt - 1))
                            nc.tensor.matmul(
                                r_ps[:, off:], ones_t[:], p[:, off:],
                                start=(jt == 0), stop=(jt == n_jt - 1))
                        rsb = p3r.tile([1, CH], F32, tag="rsb")
                        nc.vector.tensor_copy(rsb[:], r_ps[:])
                        nc.sync.dma_start(rall[h:h + 1, :], rsb[:])
                        ot = p3o.tile([HD, CH], F32R, tag="o", name=f"o{ci}_{h}")
                        nc.vector.tensor_copy(ot[:], o_ps[:])
                        o_sb.append(ot)
                    rinv = p3r.tile([HPC, CH], F32R, tag="rinv")
                    nc.vector.reciprocal(rinv[:], rall[:])
                    for h in range(HPC):
                        rrow = p3r.tile([1, CH], F32R, tag="rrow")
                        nc.sync.dma_start(rrow[:], rinv[h:h + 1, :])
                        b_ps = ps_b.tile([128, CH], F32)
                        nc.tensor.matmul(b_ps[:], onesT_t[:], rrow[:],
                                         start=True, stop=True)
                        yt = p3y.tile([HD, CH], F32R, tag="y")
                        nc.vector.tensor_mul(yt[:], o_sb[h][:], b_ps[:])
                        nc.sync.dma_start(
                            y_loc[ci][h * HD:(h + 1) * HD, :], yt[:])
                    # AllGather this chunk within the batch group (pipelines
                    # with the next chunk's attention and with the projection)
                    nc.gpsimd.collective_compute(
                        "AllGather", mybir.AluOpType.bypass,
                        replica_groups=[[0, 1, 2, 3], [4, 5, 6, 7]],
                        ins=[y_loc[ci].opt()], outs=[y_full[ci].opt()])
                    # z^T[:, ci] = Wp_slice^T @ y_full[ci] — overlaps the next
                    # chunk's attention
                    proj_chunk(ci)
    nc.compile()
    return nc


def _tables():
    inv_freq = 1.0 / (ROPE_THETA ** (np.arange(0, HD, 2, dtype=np.float64) / HD))
    pos = np.arange(S, dtype=np.float64)
    f_half = np.outer(inv_freq, pos)                  # [64, S]
    freqs = np.concatenate([f_half, f_half], axis=0)  # [HD, S]
    # match reference numerics: cos/sin computed in float32 domain
    emb32 = freqs.astype(np.float32)
    cos_t = np.cos(emb32)
    sin_t = np.sin(emb32)
    scale = np.float32(HD ** -0.5)
    sgn = np.where(np.arange(HD) < HD // 2, -1.0, 1.0).astype(np.float32)[:, None]
    cosq = (cos_t * scale).astype(np.float32)
    sinq = (sin_t * sgn * scale).astype(np.float32)
    cosk = cos_t.astype(np.float32)
    sink = (sin_t * sgn).astype(np.float32)
    return cosq, sinq, cosk, sink


_NC_CACHE = {}


def _get_nc():
    if "nc" not in _NC_CACHE:
        _NC_CACHE["nc"] = _build()
    return _NC_CACHE["nc"]


def make_in_maps(x, W_attn, W_proj):
    x = np.asarray(x, dtype=np.float32)
    W_attn = np.asarray(W_attn, dtype=np.float32)
    W_proj = np.asarray(W_proj, dtype=np.float32)
    cosq, sinq, cosk, sink = _tables()
    tri = np.triu(np.ones((128, 128), np.float32))   # [jj, ii]: keep jj <= ii
    ones = np.ones((128, 1), np.float32)
    onesT = np.ones((1, 128), np.float32)
    in_maps = []
    for c in range(N_CORES):
        b, g = divmod(c, HPC)
        in_maps.append({
            "xT": np.ascontiguousarray(x[b].T),
            "wq": np.ascontiguousarray(W_attn[:, g * EL:(g + 1) * EL]),
            "wk": np.ascontiguousarray(W_attn[:, D + g * EL:D + (g + 1) * EL]),
            "wv": np.ascontiguousarray(W_attn[:, 2 * D + g * EL:2 * D + (g + 1) * EL]),
            "wp": np.ascontiguousarray(W_proj[:, g * EL:(g + 1) * EL]),
            "cosq": cosq, "sinq": sinq, "cosk": cosk, "sink": sink,
            "tri": tri, "ones": ones, "onesT": onesT,
        })
    return in_maps


def assemble(results):
    out = np.empty((B, S, D), dtype=np.float32)
    for c in range(N_CORES):
        b, g = divmod(c, HPC)
        out[b, :, g * EL:(g + 1) * EL] = results[c]["zT"].T
    return out


def kernel(x, W_attn, W_proj):
    nc = _get_nc()
    in_maps = make_in_maps(x, W_attn, W_proj)
    res = bass_utils.run_bass_kernel_spmd(
        nc, in_maps, core_ids=list(range(N_CORES)), trace=False)
    return assemble(res.results)


if __name__ == "__main__":
    rng = np.random.default_rng(0)
    x = rng.standard_normal((B, S, D)).astype(np.float32)
    W_attn = (rng.standard_normal((D, 3 * D)) * D ** -0.5).astype(np.float32)
    W_proj = (rng.standard_normal((D, D)) * D ** -0.5).astype(np.float32)
    out = kernel(x, W_attn, W_proj)
    print("out", out.shape, out.dtype, np.abs(out).mean())


# revision 15
# speedup vs baseline: 1.0845x; 1.0845x over previous
"""Causal self-attention with RoPE on 8 TRN2 NeuronCores.

Sharding: core c -> (batch b = c//4, head-group g = c%4; 4 heads of 128 each).
Tensor-parallel over heads x data-parallel over batch. After per-head
attention, the 4 cores of a batch AllGather their y^T shards, then each
core computes a disjoint 512-column slice of the output projection.

Layouts (all chosen so no on-chip transposes are ever needed):
  xT   [D, S]   = x[b].T                      (host-transposed)
  Q^T,K^T [128, S] per head  (from matmul: lhsT=W-block, rhs=xT)
  V    [S, 512] token-major  (from matmul: lhsT=xT-tile, rhs=Wv)
  S^T  [j, i] scores blocks -> softmax sums via ones-matmul on PE
  O^T  [c, i] accumulated in PSUM, normalized by 1/rowsum afterwards
  z^T  [512, S] output slice (host transposes back)

All matmuls run in float32r (~13-bit mantissa, 4x faster than fp32 on PE).
"""
from contextlib import ExitStack

import numpy as np

import concourse.bass as bass
import concourse.tile as tile
import concourse.mybir as mybir
from concourse import bacc, bass_utils

import os as _os
B = 2
S = int(_os.environ.get("K_S", "2048"))
D = int(_os.environ.get("K_D", "2048"))
NH, HD = 16, 128
HPC = 4                 # heads per core
EL = HPC * HD           # 512: local e-width per core
CH = 512                # i-chunk / s-chunk width
NCH = S // CH           # 4
DT = D // 128           # 16 d-tiles
ROPE_THETA = 10000.0
N_CORES = 8

F32 = mybir.dt.float32
F32R = mybir.dt.float32r
AF = mybir.ActivationFunctionType


def _build():
    nc = bacc.Bacc("TRN2", target_bir_lowering=False, debug=False,
                   enable_asserts=True, num_devices=N_CORES)
    xT = nc.dram_tensor("xT", [D, S], F32R, kind="ExternalInput").ap()
    wq = nc.dram_tensor("wq", [D, EL], F32R, kind="ExternalInput").ap()
    wk = nc.dram_tensor("wk", [D, EL], F32R, kind="ExternalInput").ap()
    wv = nc.dram_tensor("wv", [D, EL], F32R, kind="ExternalInput").ap()
    wp = nc.dram_tensor("wp", [D, EL], F32R, kind="ExternalInput").ap()
    cosq = nc.dram_tensor("cosq", [HD, S], F32, kind="ExternalInput").ap()
    sinq = nc.dram_tensor("sinq", [HD, S], F32, kind="ExternalInput").ap()
    cosk = nc.dram_tensor("cosk", [HD, S], F32, kind="ExternalInput").ap()
    sink = nc.dram_tensor("sink", [HD, S], F32, kind="ExternalInput").ap()
    tri = nc.dram_tensor("tri", [128, 128], F32, kind="ExternalInput").ap()
    ones = nc.dram_tensor("ones", [128, 1], F32R, kind="ExternalInput").ap()
    onesT = nc.dram_tensor("onesT", [1, 128], F32R, kind="ExternalInput").ap()
    zT = nc.dram_tensor("zT", [EL, S], F32, kind="ExternalOutput").ap()

    xTr = xT.rearrange("(t p) s -> t p s", p=128)
    wqr = wq.rearrange("(t p) e -> t p e", p=128)
    wkr = wk.rearrange("(t p) e -> t p e", p=128)
    wvr = wv.rearrange("(t p) e -> t p e", p=128)
    wpr = wp.rearrange("(t p) e -> t p e", p=128)

    with tile.TileContext(nc) as tc, \
         nc.allow_low_precision(reason="fp32r attention"), ExitStack() as ctx:
        if True:
            vres = ctx.enter_context(tc.tile_pool(name="vres", bufs=16))
            kres = ctx.enter_context(tc.tile_pool(name="kres", bufs=4))
            cpool = ctx.enter_context(tc.tile_pool(name="const", bufs=1))
            dram = ctx.enter_context(tc.tile_pool(name="dram", bufs=1, space="DRAM"))
            ps_mm = ctx.enter_context(tc.tile_pool(name="ps_mm", bufs=2, space="PSUM"))
            ps_sc = ctx.enter_context(tc.tile_pool(name="ps_sc", bufs=3, space="PSUM"))
            ps_o = ctx.enter_context(tc.tile_pool(name="ps_o", bufs=1, space="PSUM"))
            ps_r = ctx.enter_context(tc.tile_pool(name="ps_r", bufs=1, space="PSUM"))
            ps_b = ctx.enter_context(tc.tile_pool(name="ps_b", bufs=1, space="PSUM"))

            tri_t = cpool.tile([128, 128], F32)
            nc.sync.dma_start(tri_t[:], tri)
            ones_t = cpool.tile([128, 1], F32R)
            nc.sync.dma_start(ones_t[:], ones)
            onesT_t = cpool.tile([1, 128], F32R)
            nc.sync.dma_start(onesT_t[:], onesT)

            q_spill = dram.tile([EL, S], F32R)
            y_loc = [dram.tile([EL, CH], F32R, tag=f"yl{ci}", name=f"yl{ci}")
                     for ci in range(NCH)]
            y_full = [dram.tile([D, CH], F32R, tag=f"yf{ci}", name=f"yf{ci}")
                      for ci in range(NCH)]

            v_t = [vres.tile([128, EL], F32R, tag="v", name=f"v{st}")
                   for st in range(S // 128)]
            k_t = [kres.tile([HD, S], F32R, tag="k", name=f"k{h}")
                   for h in range(HPC)]

            # ---------------- pass 1: V = x @ Wv  (token-major) -------------
            with ExitStack() as vctx:
                p1w = vctx.enter_context(tc.tile_pool(name="p1", bufs=18))
                p1x = vctx.enter_context(tc.tile_pool(name="p1x", bufs=18))
                wv_t = []
                for dt in range(DT):
                    w = p1w.tile([128, EL], F32R, tag="w", name=f"wv{dt}")
                    nc.sync.dma_start(w[:], wvr[dt])
                    wv_t.append(w)
                for sc in range(NCH):
                    xc = []
                    for dt in range(DT):
                        xt = p1x.tile([128, CH], F32R, tag="x", name=f"x{sc}_{dt}")
                        nc.sync.dma_start(xt[:], xTr[dt][:, sc * CH:(sc + 1) * CH])
                        xc.append(xt)
                    for st in range(CH // 128):
                        ps = ps_mm.tile([128, EL], F32)
                        for dt in range(DT):
                            nc.tensor.matmul(
                                ps[:], xc[dt][:, st * 128:(st + 1) * 128], wv_t[dt][:],
                                start=(dt == 0), stop=(dt == DT - 1))
                        nc.scalar.copy(v_t[sc * 4 + st][:], ps[:])

            # ------------- passes 2/3: K^T then Q^T (+RoPE), Q spilled -------
            def kq_pass(wsrc, cos_src, sin_src, is_q, tagp):
                with ExitStack() as kctx:
                    pw = kctx.enter_context(tc.tile_pool(name=f"{tagp}w", bufs=18))
                    px = kctx.enter_context(tc.tile_pool(name=f"{tagp}x", bufs=18))
                    pcs = kctx.enter_context(tc.tile_pool(name=f"{tagp}cs", bufs=2))
                    pt = kctx.enter_context(tc.tile_pool(name=f"{tagp}t", bufs=3))
                    w_t = []
                    for dt in range(DT):
                        w = pw.tile([128, EL], F32R, tag="w", name=f"{tagp}w{dt}")
                        nc.sync.dma_start(w[:], wsrc[dt])
                        w_t.append(w)
                    for sc in range(NCH):
                        xc = []
                        for dt in range(DT):
                            xt = px.tile([128, CH], F32R, tag="x",
                                         name=f"{tagp}x{sc}_{dt}")
                            nc.sync.dma_start(
                                xt[:], xTr[dt][:, sc * CH:(sc + 1) * CH])
                            xc.append(xt)
                        cs = pcs.tile([128, CH], F32, tag="cs")
                        nc.sync.dma_start(cs[:], cos_src[:, sc * CH:(sc + 1) * CH])
                        sn = pcs.tile([128, CH], F32, tag="sn")
                        nc.sync.dma_start(sn[:], sin_src[:, sc * CH:(sc + 1) * CH])
                        for h in range(HPC):
                            ps = ps_mm.tile([HD, CH], F32)
                            for dt in range(DT):
                                nc.tensor.matmul(
                                    ps[:], w_t[dt][:, h * HD:(h + 1) * HD], xc[dt][:],
                                    start=(dt == 0), stop=(dt == DT - 1))
                            pre = pt.tile([128, CH], F32, tag="pre")
                            nc.scalar.copy(pre[:], ps[:])
                            # out = pre*cos + rot(pre)*sin (sign folded into sin)
                            rot = pt.tile([128, CH], F32, tag="rot")
                            nc.sync.dma_start(rot[0:64, :], pre[64:128, :])
                            nc.sync.dma_start(rot[64:128, :], pre[0:64, :])
                            t1 = pt.tile([128, CH], F32, tag="t1")
                            nc.vector.tensor_mul(t1[:], pre[:], cs[:])
                            t2 = pt.tile([128, CH], F32, tag="rot2")
                            nc.vector.tensor_mul(t2[:], rot[:], sn[:])
                            if is_q:
                                qro = pt.tile([128, CH], F32R, tag="qro")
                                nc.vector.tensor_add(qro[:], t1[:], t2[:])
                                nc.sync.dma_start(
                                    q_spill[h * HD:(h + 1) * HD,
                                            sc * CH:(sc + 1) * CH], qro[:])
                            else:
                                nc.vector.tensor_add(
                                    k_t[h][:, sc * CH:(sc + 1) * CH], t1[:], t2[:])

            kq_pass(wkr, cosk, sink, False, "pk")
            kq_pass(wqr, cosq, sinq, True, "pq")

            # ------- pass 3: causal attention + AG + projection (pipelined) --
            with ExitStack() as actx:
                p3q = actx.enter_context(tc.tile_pool(name="p3q", bufs=5))
                p3p = actx.enter_context(tc.tile_pool(name="p3p", bufs=3))
                p3o = actx.enter_context(tc.tile_pool(name="p3o", bufs=4))
                p3y = actx.enter_context(tc.tile_pool(name="p3y", bufs=2))
                p3r = actx.enter_context(tc.tile_pool(name="p3r", bufs=2))
                p4w = actx.enter_context(tc.tile_pool(name="p4w", bufs=16))
                p4y = actx.enter_context(tc.tile_pool(name="p4y", bufs=18))
                p4z = actx.enter_context(tc.tile_pool(name="p4z", bufs=3))
                wp_t = []
                for et in range(DT):
                    w = p4w.tile([128, EL], F32R, tag="w", name=f"wp{et}")
                    nc.sync.dma_start(w[:], wpr[et])
                    wp_t.append(w)

                def proj_chunk(sc):
                    yfr = y_full[sc][:].rearrange("(t p) s -> t p s", p=128)
                    yc = []
                    for et in range(DT):
                        yt = p4y.tile([128, CH], F32R, tag="y", name=f"yg{sc}_{et}")
                        nc.sync.dma_start(yt[:], yfr[et])
                        yc.append(yt)
                    for ep in range(EL // 128):
                        ps = ps_mm.tile([128, CH], F32)
                        for et in range(DT):
                            nc.tensor.matmul(
                                ps[:], wp_t[et][:, ep * 128:(ep + 1) * 128], yc[et][:],
                                start=(et == 0), stop=(et == DT - 1))
                        zt = p4z.tile([128, CH], F32, tag="z")
                        nc.scalar.copy(zt[:], ps[:])
                        nc.sync.dma_start(
                            zT[ep * 128:(ep + 1) * 128, sc * CH:(sc + 1) * CH], zt[:])

                for ci in range(NCH):
                    qc = []
                    for h in range(HPC):
                        qt = p3q.tile([HD, CH], F32R, tag="q", name=f"q{ci}_{h}")
                        nc.sync.dma_start(
                            qt[:], q_spill[h * HD:(h + 1) * HD, ci * CH:(ci + 1) * CH])
                        qc.append(qt)
                    rall = p3r.tile([HPC, CH], F32, tag="rall")
                    o_sb = []
                    n_jt = 4 * ci + 4
                    for h in range(HPC):
                        o_ps = ps_o.tile([HD, CH], F32)
                        r_ps = ps_r.tile([1, CH], F32)
                        for jt in range(n_jt):
                            diag = jt - 4 * ci
                            off = 128 * diag if diag > 0 else 0
                            s_ps = ps_sc.tile([128, CH], F32)
                            nc.tensor.matmul(
                                s_ps[:, off:], k_t[h][:, jt * 128:(jt + 1) * 128],
                                qc[h][:, off:], start=True, stop=True)
                            p = p3p.tile([128, CH], F32R, tag="p")
                            nc.scalar.activation(p[:, off:], s_ps[:, off:], AF.Exp)
                            if diag >= 0:
                                nc.vector.tensor_mul(
                                    p[:, off:off + 128], p[:, off:off + 128], tri_t[:])
                            nc.tensor.matmul(
                                o_ps[:, off:], v_t[jt][:, h * HD:(h + 1) * HD],
                                p[:, off:], start=(jt == 0), stop=(jt == n_jt - 1))
                            nc.tensor.matmul(
                                r_ps[:, off:], ones_t[:], p[:, off:],
                                start=(jt == 0), stop=(jt == n_jt - 1))
                        rsb = p3r.tile([1, CH], F32, tag="rsb")
                        nc.vector.tensor_copy(rsb[:], r_ps[:])
                        nc.sync.dma_start(rall[h:h + 1, :], rsb[:])
                        ot = p3o.tile([HD, CH], F32R, tag="o", name=f"o{ci}_{h}")
                        nc.vector.tensor_copy(ot[:], o_ps[:])
                        o_sb.append(ot)
                    rinv = p3r.tile([HPC, CH], F32R, tag="rinv")
                    nc.vector.reciprocal(rinv[:], rall[:])
                    for h in range(HPC):
                        rrow = p3r.tile([1, CH], F32R, tag="rrow")
                        nc.sync.dma_start(rrow[:], rinv[h:h + 1, :])
                        b_ps = ps_b.tile([128, CH], F32)
                        nc.tensor.matmul(b_ps[:], onesT_t[:], rrow[:],
                                         start=True, stop=True)
                        yt = p3y.tile([HD, CH], F32R, tag="y")
                        nc.vector.tensor_mul(yt[:], o_sb[h][:], b_ps[:])
                        nc.sync.dma_start(
                            y_loc[ci][h * HD:(h + 1) * HD, :], yt[:])
                    # AllGather this chunk within the batch group (pipelines
                    # with the next chunk's attention and with the projection)
                    nc.gpsimd.collective_compute(
                        "AllGather", mybir.AluOpType.bypass,
                        replica_groups=[[0, 1, 2, 3], [4, 5, 6, 7]],
                        ins=[y_loc[ci].opt()], outs=[y_full[ci].opt()])
                # projection emitted after attention (lower scheduler priority
                # so it fills idle engine time), but pools coexist so nothing
                # forces it to wait for the attention phase to finish
                for sc in range(NCH):
                    proj_chunk(sc)
    nc.compile()
    return nc


def _tables():
    inv_freq = 1.0 / (ROPE_THETA ** (np.arange(0, HD, 2, dtype=np.float64) / HD))
    pos = np.arange(S, dtype=np.float64)
    f_half = np.outer(inv_freq, pos)                  # [64, S]
    freqs = np.concatenate([f_half, f_half], axis=0)  # [HD, S]
    # match reference numerics: cos/sin computed in float32 domain
    emb32 = freqs.astype(np.float32)
    cos_t = np.cos(emb32)
    sin_t = np.sin(emb32)
    scale = np.float32(HD ** -0.5)
    sgn = np.where(np.arange(HD) < HD // 2, -1.0, 1.0).astype(np.float32)[:, None]
    cosq = (cos_t * scale).astype(np.float32)
    sinq = (sin_t * sgn * scale).astype(np.float32)
    cosk = cos_t.astype(np.float32)
    sink = (sin_t * sgn).astype(np.float32)
    return cosq, sinq, cosk, sink


_NC_CACHE = {}


def _get_nc():
    if "nc" not in _NC_CACHE:
        _NC_CACHE["nc"] = _build()
    return _NC_CACHE["nc"]


def make_in_maps(x, W_attn, W_proj):
    x = np.asarray(x, dtype=np.float32)
    W_attn = np.asarray(W_attn, dtype=np.float32)
    W_proj = np.asarray(W_proj, dtype=np.float32)
    cosq, sinq, cosk, sink = _tables()
    tri = np.triu(np.ones((128, 128), np.float32))   # [jj, ii]: keep jj <= ii
    ones = np.ones((128, 1), np.float32)
    onesT = np.ones((1, 128), np.float32)
    in_maps = []
    for c in range(N_CORES):
        b, g = divmod(c, HPC)
        in_maps.append({
            "xT": np.ascontiguousarray(x[b].T),
            "wq": np.ascontiguousarray(W_attn[:, g * EL:(g + 1) * EL]),
            "wk": np.ascontiguousarray(W_attn[:, D + g * EL:D + (g + 1) * EL]),
            "wv": np.ascontiguousarray(W_attn[:, 2 * D + g * EL:2 * D + (g + 1) * EL]),
            "wp": np.ascontiguousarray(W_proj[:, g * EL:(g + 1) * EL]),
            "cosq": cosq, "sinq": sinq, "cosk": cosk, "sink": sink,
            "tri": tri, "ones": ones, "onesT": onesT,
        })
    return in_maps


def assemble(results):
    out = np.empty((B, S, D), dtype=np.float32)
    for c in range(N_CORES):
        b, g = divmod(c, HPC)
        out[b, :, g * EL:(g + 1) * EL] = results[c]["zT"].T
    return out


def kernel(x, W_attn, W_proj):
    nc = _get_nc()
    in_maps = make_in_maps(x, W_attn, W_proj)
    res = bass_utils.run_bass_kernel_spmd(
        nc, in_maps, core_ids=list(range(N_CORES)), trace=False)
    return assemble(res.results)


if __name__ == "__main__":
    rng = np.random.default_rng(0)
    x = rng.standard_normal((B, S, D)).astype(np.float32)
    W_attn = (rng.standard_normal((D, 3 * D)) * D ** -0.5).astype(np.float32)
    W_proj = (rng.standard_normal((D, D)) * D ** -0.5).astype(np.float32)
    out = kernel(x, W_attn, W_proj)
    print("out", out.shape, out.dtype, np.abs(out).mean())
